# revision 20
# baseline (speedup 1.0000x reference)
"""Trainium2 Bass kernel for nn_FCAutoEncoder (ragged_sequence).

Strategy:
  * Host: bucket rows by seq_length (5 sizes), split each bucket evenly
    across 8 cores (pure data parallel), transpose to feature-major
    [1024, R] per core so activations live as [feat_part, batch_free].
    All feature dims are zero-padded to multiples of 128 so every
    matmul K-tile is a full 128 partitions (partial-K matmuls measure
    ~2.5x slower on HW).
  * Device (per core, identical SPMD program): per bucket k, per column
    chunk (<=512, even): expand with Win[k] restricted to its true s_k
    input features, shared 1008-512-256-128-256-512-1008 MLP, contract
    with Wout[k] restricted to s_k output features.  All matmuls run as
    float32r (full-rate fp32 path, ~2e-4 rel err, fp32 PSUM accum).
    PSUM is evacuated with fused bias(+ReLU) on ScalarE/VectorE.
    Weights stream in one batched DMA per tensor, in first-use order,
    with one-bucket-ahead prefetch so the PE never waits on HBM.
  * Host: transpose back, scatter rows to original order; rows beyond
    s_k and rows with unknown lengths are zero.
"""
import os
import sys

sys.path.insert(0, "/opt/trn_rl_repo")

import numpy as np

SIZES = (36, 72, 144, 288, 1008)
SP = (128, 128, 256, 384, 1024)   # SIZES padded to multiples of 128
BASE = 1008
BASE_P = 1024
H1, H2, LAT = 512, 256, 128
N_CORES = 8
MAX_CHUNK = 448
ACT_BUFS = 33

_last_exec_ns = None
_prog_cache = {}


def _tiles(n, t=128):
    return [(s, min(t, n - s)) for s in range(0, n, t)]


def _chunks(c, maxn=MAX_CHUNK):
    """Split c (even) into even-sized chunks <= maxn.

    float32r matmuls require an even moving dim, so every chunk is even.
    """
    if c <= 0:
        return []
    assert c % 2 == 0
    half = c // 2
    n = (c + maxn - 1) // maxn
    base, rem = divmod(half, n)
    out, off = [], 0
    for i in range(n):
        sz = 2 * (base + (1 if i < rem else 0))
        out.append((off, sz))
        off += sz
    return out


def _bias_layout():
    """Fixed column order of the packed [128, NB] bias tensor."""
    cols = []
    for k in range(5):
        for (ms, mp) in _tiles(BASE_P):
            cols.append(("exp", k, ms, mp))
    for (js, jp) in _tiles(H1):
        cols.append(("L1", 0, js, jp))
    for (js, jp) in _tiles(H2):
        cols.append(("L2", 0, js, jp))
    for (js, jp) in _tiles(LAT):
        cols.append(("L3", 0, js, jp))
    for (js, jp) in _tiles(H2):
        cols.append(("D1", 0, js, jp))
    for (js, jp) in _tiles(H1):
        cols.append(("D2", 0, js, jp))
    for (ms, mp) in _tiles(BASE_P):
        cols.append(("D3", 0, ms, mp))
    for k in range(5):
        for (os_, op) in _tiles(SIZES[k]):
            cols.append(("out", k, os_, op))
    return cols


def _build_program(c_ks, R):
    import concourse.bacc as bacc
    import concourse.mybir as mybir
    from concourse import tile

    f32 = mybir.dt.float32
    f32r = mybir.dt.float32r
    AF = mybir.ActivationFunctionType
    ALU = mybir.AluOpType

    bias_cols = _bias_layout()
    bias_idx = {c[:3]: i for i, c in enumerate(bias_cols)}

    def bcol(layer, k, start):
        return bias_idx[(layer, k, start)]

    nc = bacc.Bacc(None, target_bir_lowering=False, debug=False, num_devices=1)

    xT = nc.dram_tensor("xT", [BASE_P, R], f32, kind="ExternalInput").ap()
    outT = nc.dram_tensor("outT", [BASE, R], f32, kind="ExternalOutput").ap()
    winT = [
        nc.dram_tensor(f"winT{k}", [SP[k], BASE_P], f32, kind="ExternalInput").ap()
        for k in range(5)
    ]
    woutT = [
        nc.dram_tensor(f"woutT{k}", [BASE_P, SIZES[k]], f32,
                       kind="ExternalInput").ap()
        for k in range(5)
    ]
    we1T = nc.dram_tensor("we1T", [BASE_P, H1], f32, kind="ExternalInput").ap()
    we2T = nc.dram_tensor("we2T", [H1, H2], f32, kind="ExternalInput").ap()
    we3T = nc.dram_tensor("we3T", [H2, LAT], f32, kind="ExternalInput").ap()
    wd1T = nc.dram_tensor("wd1T", [LAT, H2], f32, kind="ExternalInput").ap()
    wd2T = nc.dram_tensor("wd2T", [H2, H1], f32, kind="ExternalInput").ap()
    wd3T = nc.dram_tensor("wd3T", [H1, BASE_P], f32, kind="ExternalInput").ap()
    biasD = nc.dram_tensor("biases", [128, len(bias_cols)], f32,
                           kind="ExternalInput").ap()

    with tile.TileContext(nc) as tc:
        with (
            tc.tile_pool(name="wp", bufs=1) as wp,
            tc.tile_pool(name="ap", bufs=ACT_BUFS) as apool,
            tc.tile_pool(name="pp", bufs=8, space="PSUM") as pp,
        ):
            bias_t = wp.tile([128, len(bias_cols)], f32, tag="bias")
            bias_loaded = [False]

            def load_w(dram, n_rows, n_cols, tag, col_split=None):
                """One batched DMA: [t*128, C] dram -> [128, t, C] tile.

                col_split: issue several DMAs over column ranges so early
                consumers (first expand M-tiles) start sooner.
                """
                t = n_rows // 128
                tl = wp.tile([128, t, n_cols], f32r, tag=tag)
                r = dram.rearrange("(t p) c -> p t c", p=128).bitcast(f32r)
                if col_split:
                    for cs in range(0, n_cols, col_split):
                        ce = min(cs + col_split, n_cols)
                        nc.sync.dma_start(tl[:, :, cs:ce], r[:, :, cs:ce])
                else:
                    nc.sync.dma_start(tl[:], r)
                return tl

            win_t = {}
            wout_t = {}
            mlp_t = {}

            def mlp_load(part):
                if part in mlp_t:
                    return
                srcs = {"we1": (we1T, BASE_P, H1), "we2": (we2T, H1, H2),
                        "we3": (we3T, H2, LAT), "wd1": (wd1T, LAT, H2),
                        "wd2": (wd2T, H2, H1), "wd3": (wd3T, H1, BASE_P)}
                d, a, b = srcs[part]
                mlp_t[part] = load_w(d, a, b, part)

            def mlp_weights():
                for p in ("we1", "we2", "we3", "wd1", "wd2", "wd3"):
                    mlp_load(p)
                return mlp_t

            evac_rr = [0]

            def evac(psum, mp, cn, bias_j, relu, eng, out_dt):
                pass  # engine chosen statically per layer (or alternated)
                o = apool.tile([mp, cn], out_dt, tag="act")
                b = bias_t[:mp, bias_j:bias_j + 1]
                if eng == "act":
                    nc.scalar.activation(
                        o[:], psum[:], AF.Relu if relu else AF.Identity, bias=b
                    )
                else:
                    if relu:
                        nc.vector.tensor_scalar(
                            o[:], psum[:], b, 0.0, ALU.add, ALU.max
                        )
                    else:
                        nc.vector.tensor_scalar_add(o[:], psum[:], b)
                return o

            def layer(in_tiles, wtile, n_in, n_out, bias_layer, bias_k,
                      relu, eng, cn, out_dt=f32r):
                outs = []
                nkt = n_in // 128
                for (js, jp) in _tiles(n_out):
                    psum = pp.tile([jp, cn], f32, tag="ps")
                    for i in range(nkt):
                        nc.tensor.matmul(
                            psum[:], wtile[:, i, js:js + jp], in_tiles[i][:],
                            start=(i == 0), stop=(i == nkt - 1),
                        )
                    e_i = ("dve" if (js // 128) % 2 == 0 else "act") \
                        if eng == "alt" else eng
                    outs.append(
                        evac(psum, jp, cn, bcol(bias_layer, bias_k, js),
                             relu, e_i, out_dt)
                    )
                return outs

            def load_x(k, g0, cn):
                xts = []
                for (ks, kp) in _tiles(SP[k]):
                    t = apool.tile([kp, cn], f32r, tag="act")
                    nc.sync.dma_start(
                        t[:], xT[ks:ks + kp, g0:g0 + cn].bitcast(f32r)
                    )
                    xts.append(t)
                return xts

            # largest bucket first: its long expand covers the MLP weight
            # loads, and the run ends on a small contract
            buckets = sorted((k for k in range(5) if c_ks[k] > 0),
                             key=lambda k: -SP[k])
            offs = {}
            off = 0
            for k in range(5):
                offs[k] = off
                off += c_ks[k]

            xpre = {}

            def sub_layer(in_tiles, wtile, n_in, jtl, bias_layer,
                          relu, eng, cn):
                outs = []
                nkt = n_in // 128
                for (js, jp) in jtl:
                    psum = pp.tile([jp, cn], f32, tag="ps")
                    for i in range(nkt):
                        nc.tensor.matmul(
                            psum[:], wtile[:, i, js:js + jp], in_tiles[i][:],
                            start=(i == 0), stop=(i == nkt - 1),
                        )
                    e_i = ("dve" if (js // 128) % 2 == 0 else "act") \
                        if eng == "alt" else eng
                    outs.append(
                        evac(psum, jp, cn, bcol(bias_layer, 0, js),
                             relu, e_i, f32r)
                    )
                return outs

            def emit_contract(k, g0, cn, dec):
                for (os_, op) in _tiles(SIZES[k]):
                    psum = pp.tile([op, cn], f32, tag="ps")
                    for i in range(BASE_P // 128):
                        nc.tensor.matmul(
                            psum[:], wout_t[k][:, i, os_:os_ + op],
                            dec[i][:],
                            start=(i == 0), stop=(i == BASE_P // 128 - 1),
                        )
                    ot = evac(psum, op, cn, bcol("out", k, os_),
                              False, "act", f32)
                    nc.sync.dma_start(
                        outT[os_:os_ + op, g0:g0 + cn], ot[:]
                    )

            def tail_stages(k, g0, cn, h2, w):
                """Generator of tail stages; caller interleaves them."""
                lat = sub_layer(h2, w["we3"], H2, _tiles(LAT), "L3",
                                False, "dve", cn)
                yield
                d1 = sub_layer(lat, w["wd1"], LAT, _tiles(H2), "D1",
                               True, "dve", cn)
                yield
                d2 = sub_layer(d1, w["wd2"], H2, _tiles(H1), "D2",
                               True, "act", cn)
                yield
                dec = sub_layer(d2, w["wd3"], H1, _tiles(BASE_P), "D3",
                                False, "alt", cn)
                yield
                emit_contract(k, g0, cn, dec)

            # units: (bucket, chunk_start, chunk_len) in processing order
            units = []
            for k in buckets:
                for (c0, cn) in _chunks(c_ks[k]):
                    units.append((k, offs[k] + c0, cn))

            tail_prev = None
            for ui, (k, g0, cn) in enumerate(units):
                s_k = SIZES[k]
                first = ui == 0
                nxt = units[ui + 1] if ui + 1 < len(units) else None
                if k not in win_t:
                    win_t[k] = load_w(winT[k], SP[k], BASE_P, f"win{k}",
                                      col_split=256 if first else None)
                if not bias_loaded[0]:
                    nc.sync.dma_start(bias_t[:], biasD[:])
                    bias_loaded[0] = True
                xts = xpre.pop((k, g0), None) or load_x(k, g0, cn)
                e = layer(xts, win_t[k], SP[k], BASE_P, "exp", k,
                          False, "alt", cn)
                # weight DMAs in need-order: L1 weights first, then next
                # unit's expand inputs, then the rest
                mlp_load("we1")
                if first and nxt is not None:
                    nk, ng0, ncn = nxt
                    xpre[(nk, ng0)] = load_x(nk, ng0, ncn)
                if nxt is not None:
                    nk, ng0, ncn = nxt
                    if nk not in win_t:
                        win_t[nk] = load_w(winT[nk], SP[nk], BASE_P,
                                           f"win{nk}")
                mlp_load("we2")
                mlp_load("we3")
                jt1 = _tiles(H1)
                if tail_prev is not None:
                    next(tail_prev, None)              # L3(prev)
                h1a = sub_layer(e, mlp_t["we1"], BASE_P, jt1[:2], "L1",
                                True, "act", cn)
                mlp_load("wd1")
                mlp_load("wd2")
                if tail_prev is not None:
                    next(tail_prev, None)              # D1(prev)
                h1b = sub_layer(e, mlp_t["we1"], BASE_P, jt1[2:], "L1",
                                True, "act", cn)
                mlp_load("wd3")
                if k not in wout_t:
                    wout_t[k] = load_w(woutT[k], BASE_P, s_k, f"wout{k}")
                if tail_prev is not None:
                    next(tail_prev, None)              # D2(prev)
                if not first and nxt is not None:
                    nk, ng0, ncn = nxt
                    xpre[(nk, ng0)] = load_x(nk, ng0, ncn)
                h1 = h1a + h1b
                h2 = sub_layer(h1, mlp_t["we2"], H1, _tiles(H2), "L2",
                               True, "act", cn)
                if nxt is not None:
                    nk, ng0, ncn = nxt
                    if nk not in wout_t:
                        wout_t[nk] = load_w(woutT[nk], BASE_P, SIZES[nk],
                                            f"wout{nk}")
                if tail_prev is not None:
                    next(tail_prev, None)              # D3(prev)
                    next(tail_prev, None)              # contract(prev)
                tail_prev = tail_stages(k, g0, cn, h2, mlp_t)

            if tail_prev is not None:
                for _ in tail_prev:
                    pass

    nc.compile()
    return nc


def _pad(a, shape):
    out = np.zeros(shape, dtype=np.float32)
    out[tuple(slice(0, s) for s in a.shape)] = a
    return out


def kernel(**inputs):
    global _last_exec_ns
    from concourse.bass_utils import run_bass_kernel_spmd

    x = np.asarray(inputs["x"], dtype=np.float32)
    seq = np.asarray(inputs["seq_lengths"]).astype(np.int64)
    B = x.shape[0]

    Win = np.asarray(inputs["Win"], dtype=np.float32)
    bin_ = np.asarray(inputs["bin_"], dtype=np.float32)
    Wout = np.asarray(inputs["Wout"], dtype=np.float32)
    bout = np.asarray(inputs["bout"], dtype=np.float32)
    We1 = np.asarray(inputs["We1"], dtype=np.float32)
    be1 = np.asarray(inputs["be1"], dtype=np.float32)
    We2 = np.asarray(inputs["We2"], dtype=np.float32)
    be2 = np.asarray(inputs["be2"], dtype=np.float32)
    We3 = np.asarray(inputs["We3"], dtype=np.float32)
    be3 = np.asarray(inputs["be3"], dtype=np.float32)
    Wd1 = np.asarray(inputs["Wd1"], dtype=np.float32)
    bd1 = np.asarray(inputs["bd1"], dtype=np.float32)
    Wd2 = np.asarray(inputs["Wd2"], dtype=np.float32)
    bd2 = np.asarray(inputs["bd2"], dtype=np.float32)
    Wd3 = np.asarray(inputs["Wd3"], dtype=np.float32)
    bd3 = np.asarray(inputs["bd3"], dtype=np.float32)

    # ---- bucket rows by size ----
    idx = [np.nonzero(seq == s)[0] for s in SIZES]
    n_ks = [len(i) for i in idx]
    # even-rounded per-core counts (float32r needs even moving dims)
    c_ks = tuple(2 * (-(-n // (2 * N_CORES))) if n > 0 else 0 for n in n_ks)
    R = sum(c_ks)

    out = np.zeros((B, BASE), dtype=np.float32)
    if R == 0:
        return out

    offs = np.cumsum([0] + list(c_ks))[:-1]

    # ---- shared (replicated) weight inputs, padded to 128-multiples ----
    shared = {}
    for k in range(5):
        s = SIZES[k]
        shared[f"winT{k}"] = _pad(Win[k].T[:s, :], (SP[k], BASE_P))
        shared[f"woutT{k}"] = _pad(Wout[k].T[:, :s], (BASE_P, s))
    shared["we1T"] = _pad(We1.T, (BASE_P, H1))
    shared["we2T"] = np.ascontiguousarray(We2.T)
    shared["we3T"] = np.ascontiguousarray(We3.T)
    shared["wd1T"] = np.ascontiguousarray(Wd1.T)
    shared["wd2T"] = np.ascontiguousarray(Wd2.T)
    shared["wd3T"] = _pad(Wd3.T, (H1, BASE_P))

    bias_cols = _bias_layout()
    bp = np.zeros((128, len(bias_cols)), dtype=np.float32)
    vecs = {"L1": be1, "L2": be2, "L3": be3, "D1": bd1, "D2": bd2, "D3": bd3}
    for j, col in enumerate(bias_cols):
        layer, k, start, width = col
        if layer == "exp":
            v = bin_[k][start:start + width]
        elif layer == "out":
            v = bout[k][start:start + width]
        else:
            v = vecs[layer][start:start + width]
        bp[: len(v), j] = v
    shared["biases"] = bp

    # ---- per-core inputs ----
    in_maps = []
    core_rows = []
    for m in range(N_CORES):
        Xc = np.zeros((R, BASE_P), dtype=np.float32)
        rows_info = []
        for k in range(5):
            if c_ks[k] == 0:
                continue
            lo = m * c_ks[k]
            rows = idx[k][lo:lo + c_ks[k]]
            if len(rows):
                Xc[offs[k]:offs[k] + len(rows), :BASE] = x[rows]
            rows_info.append((k, rows, offs[k]))
        in_maps.append({"xT": np.ascontiguousarray(Xc.T), **shared})
        core_rows.append(rows_info)

    # ---- build / fetch program ----
    key = (c_ks, R)
    if key not in _prog_cache:
        _prog_cache[key] = _build_program(c_ks, R)
    nc = _prog_cache[key]

    trace = bool(os.environ.get("BASS_TRACE"))
    res = run_bass_kernel_spmd(nc, in_maps, list(range(N_CORES)), trace=trace)
    _last_exec_ns = res.exec_time_ns

    # ---- gather / unsort ----
    for m in range(N_CORES):
        oT = res.results[m]["outT"]
        for (k, rows, o) in core_rows[m]:
            if len(rows):
                out[rows] = oT[:, o:o + len(rows)].T
    return out


# revision 21
# speedup vs baseline: 1.0166x; 1.0166x over previous
"""Trainium2 Bass kernel for nn_FCAutoEncoder (ragged_sequence).

Strategy:
  * Host: bucket rows by seq_length (5 sizes), split each bucket evenly
    across 8 cores (pure data parallel), transpose to feature-major
    [1024, R] per core so activations live as [feat_part, batch_free].
    All feature dims are zero-padded to multiples of 128 so every
    matmul K-tile is a full 128 partitions (partial-K matmuls measure
    ~2.5x slower on HW).
  * Device (per core, identical SPMD program): per bucket k, per column
    chunk (<=512, even): expand with Win[k] restricted to its true s_k
    input features, shared 1008-512-256-128-256-512-1008 MLP, contract
    with Wout[k] restricted to s_k output features.  All matmuls run as
    float32r (full-rate fp32 path, ~2e-4 rel err, fp32 PSUM accum).
    PSUM is evacuated with fused bias(+ReLU) on ScalarE/VectorE.
    Weights stream in one batched DMA per tensor, in first-use order,
    with one-bucket-ahead prefetch so the PE never waits on HBM.
  * Host: transpose back, scatter rows to original order; rows beyond
    s_k and rows with unknown lengths are zero.
"""
import os
import sys

sys.path.insert(0, "/opt/trn_rl_repo")

import numpy as np

SIZES = (36, 72, 144, 288, 1008)
SP = (128, 128, 256, 384, 1024)   # SIZES padded to multiples of 128
BASE = 1008
BASE_P = 1024
H1, H2, LAT = 512, 256, 128
N_CORES = 8
MAX_CHUNK = 448
ACT_BUFS = 33

_last_exec_ns = None
_prog_cache = {}


def _tiles(n, t=128):
    return [(s, min(t, n - s)) for s in range(0, n, t)]


def _chunks(c, maxn=MAX_CHUNK):
    """Split c (even) into even-sized chunks <= maxn.

    float32r matmuls require an even moving dim, so every chunk is even.
    """
    if c <= 0:
        return []
    assert c % 2 == 0
    half = c // 2
    n = (c + maxn - 1) // maxn
    base, rem = divmod(half, n)
    out, off = [], 0
    for i in range(n):
        sz = 2 * (base + (1 if i < rem else 0))
        out.append((off, sz))
        off += sz
    return out


def _bias_layout():
    """Fixed column order of the packed [128, NB] bias tensor."""
    cols = []
    for k in range(5):
        for (ms, mp) in _tiles(BASE_P):
            cols.append(("exp", k, ms, mp))
    for (js, jp) in _tiles(H1):
        cols.append(("L1", 0, js, jp))
    for (js, jp) in _tiles(H2):
        cols.append(("L2", 0, js, jp))
    for (js, jp) in _tiles(LAT):
        cols.append(("L3", 0, js, jp))
    for (js, jp) in _tiles(H2):
        cols.append(("D1", 0, js, jp))
    for (js, jp) in _tiles(H1):
        cols.append(("D2", 0, js, jp))
    for (ms, mp) in _tiles(BASE_P):
        cols.append(("D3", 0, ms, mp))
    for k in range(5):
        for (os_, op) in _tiles(SIZES[k]):
            cols.append(("out", k, os_, op))
    return cols


def _build_program(c_ks, R):
    import concourse.bacc as bacc
    import concourse.mybir as mybir
    from concourse import tile

    f32 = mybir.dt.float32
    f32r = mybir.dt.float32r
    AF = mybir.ActivationFunctionType
    ALU = mybir.AluOpType

    bias_cols = _bias_layout()
    bias_idx = {c[:3]: i for i, c in enumerate(bias_cols)}

    def bcol(layer, k, start):
        return bias_idx[(layer, k, start)]

    nc = bacc.Bacc(None, target_bir_lowering=False, debug=False, num_devices=1)

    xT = nc.dram_tensor("xT", [BASE_P, R], f32, kind="ExternalInput").ap()
    outT = nc.dram_tensor("outT", [BASE, R], f32, kind="ExternalOutput").ap()
    winT = [
        nc.dram_tensor(f"winT{k}", [SP[k], BASE_P], f32, kind="ExternalInput").ap()
        for k in range(5)
    ]
    woutT = [
        nc.dram_tensor(f"woutT{k}", [BASE_P, SIZES[k]], f32,
                       kind="ExternalInput").ap()
        for k in range(5)
    ]
    we1T = nc.dram_tensor("we1T", [BASE_P, H1], f32, kind="ExternalInput").ap()
    we2T = nc.dram_tensor("we2T", [H1, H2], f32, kind="ExternalInput").ap()
    we3T = nc.dram_tensor("we3T", [H2, LAT], f32, kind="ExternalInput").ap()
    wd1T = nc.dram_tensor("wd1T", [LAT, H2], f32, kind="ExternalInput").ap()
    wd2T = nc.dram_tensor("wd2T", [H2, H1], f32, kind="ExternalInput").ap()
    wd3T = nc.dram_tensor("wd3T", [H1, BASE_P], f32, kind="ExternalInput").ap()
    biasD = nc.dram_tensor("biases", [128, len(bias_cols)], f32,
                           kind="ExternalInput").ap()

    with tile.TileContext(nc) as tc:
        with (
            tc.tile_pool(name="wp", bufs=1) as wp,
            tc.tile_pool(name="ap", bufs=ACT_BUFS) as apool,
            tc.tile_pool(name="pp", bufs=8, space="PSUM") as pp,
        ):
            bias_t = wp.tile([128, len(bias_cols)], f32, tag="bias")
            bias_loaded = [False]

            def load_w(dram, n_rows, n_cols, tag, col_split=None):
                """One batched DMA: [t*128, C] dram -> [128, t, C] tile.

                col_split: issue several DMAs over column ranges so early
                consumers (first expand M-tiles) start sooner.
                """
                t = n_rows // 128
                tl = wp.tile([128, t, n_cols], f32r, tag=tag)
                r = dram.rearrange("(t p) c -> p t c", p=128).bitcast(f32r)
                if col_split:
                    for cs in range(0, n_cols, col_split):
                        ce = min(cs + col_split, n_cols)
                        nc.sync.dma_start(tl[:, :, cs:ce], r[:, :, cs:ce])
                else:
                    nc.sync.dma_start(tl[:], r)
                return tl

            win_t = {}
            wout_t = {}
            mlp_t = {}

            def mlp_load(part):
                if part in mlp_t:
                    return
                srcs = {"we1": (we1T, BASE_P, H1), "we2": (we2T, H1, H2),
                        "we3": (we3T, H2, LAT), "wd1": (wd1T, LAT, H2),
                        "wd2": (wd2T, H2, H1), "wd3": (wd3T, H1, BASE_P)}
                d, a, b = srcs[part]
                mlp_t[part] = load_w(d, a, b, part)

            def mlp_weights():
                for p in ("we1", "we2", "we3", "wd1", "wd2", "wd3"):
                    mlp_load(p)
                return mlp_t

            evac_rr = [0]

            def evac(psum, mp, cn, bias_j, relu, eng, out_dt):
                pass  # engine chosen statically per layer (or alternated)
                o = apool.tile([mp, cn], out_dt, tag="act")
                b = bias_t[:mp, bias_j:bias_j + 1]
                if eng == "act":
                    nc.scalar.activation(
                        o[:], psum[:], AF.Relu if relu else AF.Identity, bias=b
                    )
                else:
                    if relu:
                        nc.vector.tensor_scalar(
                            o[:], psum[:], b, 0.0, ALU.add, ALU.max
                        )
                    else:
                        nc.vector.tensor_scalar_add(o[:], psum[:], b)
                return o

            def layer(in_tiles, wtile, n_in, n_out, bias_layer, bias_k,
                      relu, eng, cn, out_dt=f32r):
                outs = []
                nkt = n_in // 128
                for (js, jp) in _tiles(n_out):
                    psum = pp.tile([jp, cn], f32, tag="ps")
                    for i in range(nkt):
                        nc.tensor.matmul(
                            psum[:], wtile[:, i, js:js + jp], in_tiles[i][:],
                            start=(i == 0), stop=(i == nkt - 1),
                        )
                    e_i = ("dve" if (js // 128) % 2 == 0 else "act") \
                        if eng == "alt" else eng
                    outs.append(
                        evac(psum, jp, cn, bcol(bias_layer, bias_k, js),
                             relu, e_i, out_dt)
                    )
                return outs

            def load_x(k, g0, cn):
                xts = []
                for (ks, kp) in _tiles(SP[k]):
                    t = apool.tile([kp, cn], f32r, tag="act")
                    nc.sync.dma_start(
                        t[:], xT[ks:ks + kp, g0:g0 + cn].bitcast(f32r)
                    )
                    xts.append(t)
                return xts

            # largest bucket first: its long expand covers the MLP weight
            # loads, and the run ends on a small contract
            buckets = sorted((k for k in range(5) if c_ks[k] > 0),
                             key=lambda k: -SP[k])
            offs = {}
            off = 0
            for k in range(5):
                offs[k] = off
                off += c_ks[k]

            xpre = {}

            def sub_layer(in_tiles, wtile, n_in, jtl, bias_layer,
                          relu, eng, cn):
                outs = []
                nkt = n_in // 128
                for (js, jp) in jtl:
                    psum = pp.tile([jp, cn], f32, tag="ps")
                    for i in range(nkt):
                        nc.tensor.matmul(
                            psum[:], wtile[:, i, js:js + jp], in_tiles[i][:],
                            start=(i == 0), stop=(i == nkt - 1),
                        )
                    e_i = ("dve" if (js // 128) % 2 == 0 else "act") \
                        if eng == "alt" else eng
                    outs.append(
                        evac(psum, jp, cn, bcol(bias_layer, 0, js),
                             relu, e_i, f32r)
                    )
                return outs

            def emit_contract(k, g0, cn, dec):
                for (os_, op) in _tiles(SIZES[k]):
                    psum = pp.tile([op, cn], f32, tag="ps")
                    for i in range(BASE_P // 128):
                        nc.tensor.matmul(
                            psum[:], wout_t[k][:, i, os_:os_ + op],
                            dec[i][:],
                            start=(i == 0), stop=(i == BASE_P // 128 - 1),
                        )
                    ot = evac(psum, op, cn, bcol("out", k, os_),
                              False, "act", f32)
                    nc.sync.dma_start(
                        outT[os_:os_ + op, g0:g0 + cn], ot[:]
                    )

            def tail_stages(k, g0, cn, h2, w):
                """Generator of tail stages; caller interleaves them."""
                lat = sub_layer(h2, w["we3"], H2, _tiles(LAT), "L3",
                                False, "dve", cn)
                yield
                d1 = sub_layer(lat, w["wd1"], LAT, _tiles(H2), "D1",
                               True, "dve", cn)
                yield
                d2 = sub_layer(d1, w["wd2"], H2, _tiles(H1), "D2",
                               True, "act", cn)
                yield
                dec = sub_layer(d2, w["wd3"], H1, _tiles(BASE_P), "D3",
                                False, "alt", cn)
                yield
                emit_contract(k, g0, cn, dec)

            # units: (bucket, chunk_start, chunk_len) in processing order
            units = []
            for k in buckets:
                for (c0, cn) in _chunks(c_ks[k]):
                    units.append((k, offs[k] + c0, cn))

            tail_prev = None
            for ui, (k, g0, cn) in enumerate(units):
                s_k = SIZES[k]
                first = ui == 0
                nxt = units[ui + 1] if ui + 1 < len(units) else None
                if first:
                    # interleave x K-tile and Win column-chunk DMAs 1:1 so
                    # the first expand matmul starts ~2us after DMA start
                    t = SP[k] // 128
                    wt = wp.tile([128, t, BASE_P], f32r, tag=f"win{k}")
                    wr = winT[k].rearrange("(t p) c -> p t c",
                                           p=128).bitcast(f32r)
                    xts = []
                    for i, (ks, kp) in enumerate(_tiles(SP[k])):
                        xt = apool.tile([kp, cn], f32r, tag="act")
                        nc.sync.dma_start(
                            xt[:], xT[ks:ks + kp, g0:g0 + cn].bitcast(f32r)
                        )
                        xts.append(xt)
                        cs, ce = i * 128, (i + 1) * 128
                        nc.sync.dma_start(wt[:, :, cs:ce], wr[:, :, cs:ce])
                    for i in range(t, BASE_P // 128):
                        cs, ce = i * 128, (i + 1) * 128
                        nc.sync.dma_start(wt[:, :, cs:ce], wr[:, :, cs:ce])
                    win_t[k] = wt
                    nc.sync.dma_start(bias_t[:], biasD[:])
                    bias_loaded[0] = True
                else:
                    if k not in win_t:
                        win_t[k] = load_w(winT[k], SP[k], BASE_P, f"win{k}")
                    xts = xpre.pop((k, g0), None) or load_x(k, g0, cn)
                e = layer(xts, win_t[k], SP[k], BASE_P, "exp", k,
                          False, "alt", cn)
                # weight DMAs in need-order: L1 weights first, then next
                # unit's expand inputs, then the rest
                mlp_load("we1")
                if first and nxt is not None:
                    nk, ng0, ncn = nxt
                    xpre[(nk, ng0)] = load_x(nk, ng0, ncn)
                if nxt is not None:
                    nk, ng0, ncn = nxt
                    if nk not in win_t:
                        win_t[nk] = load_w(winT[nk], SP[nk], BASE_P,
                                           f"win{nk}")
                mlp_load("we2")
                mlp_load("we3")
                jt1 = _tiles(H1)
                if tail_prev is not None:
                    next(tail_prev, None)              # L3(prev)
                h1a = sub_layer(e, mlp_t["we1"], BASE_P, jt1[:2], "L1",
                                True, "act", cn)
                mlp_load("wd1")
                mlp_load("wd2")
                if tail_prev is not None:
                    next(tail_prev, None)              # D1(prev)
                h1b = sub_layer(e, mlp_t["we1"], BASE_P, jt1[2:], "L1",
                                True, "act", cn)
                mlp_load("wd3")
                if k not in wout_t:
                    wout_t[k] = load_w(woutT[k], BASE_P, s_k, f"wout{k}")
                if tail_prev is not None:
                    next(tail_prev, None)              # D2(prev)
                if not first and nxt is not None:
                    nk, ng0, ncn = nxt
                    xpre[(nk, ng0)] = load_x(nk, ng0, ncn)
                h1 = h1a + h1b
                h2 = sub_layer(h1, mlp_t["we2"], H1, _tiles(H2), "L2",
                               True, "act", cn)
                if nxt is not None:
                    nk, ng0, ncn = nxt
                    if nk not in wout_t:
                        wout_t[nk] = load_w(woutT[nk], BASE_P, SIZES[nk],
                                            f"wout{nk}")
                if tail_prev is not None:
                    next(tail_prev, None)              # D3(prev)
                    next(tail_prev, None)              # contract(prev)
                tail_prev = tail_stages(k, g0, cn, h2, mlp_t)

            if tail_prev is not None:
                for _ in tail_prev:
                    pass

    nc.compile()
    return nc


def _pad(a, shape):
    out = np.zeros(shape, dtype=np.float32)
    out[tuple(slice(0, s) for s in a.shape)] = a
    return out


def kernel(**inputs):
    global _last_exec_ns
    from concourse.bass_utils import run_bass_kernel_spmd

    x = np.asarray(inputs["x"], dtype=np.float32)
    seq = np.asarray(inputs["seq_lengths"]).astype(np.int64)
    B = x.shape[0]

    Win = np.asarray(inputs["Win"], dtype=np.float32)
    bin_ = np.asarray(inputs["bin_"], dtype=np.float32)
    Wout = np.asarray(inputs["Wout"], dtype=np.float32)
    bout = np.asarray(inputs["bout"], dtype=np.float32)
    We1 = np.asarray(inputs["We1"], dtype=np.float32)
    be1 = np.asarray(inputs["be1"], dtype=np.float32)
    We2 = np.asarray(inputs["We2"], dtype=np.float32)
    be2 = np.asarray(inputs["be2"], dtype=np.float32)
    We3 = np.asarray(inputs["We3"], dtype=np.float32)
    be3 = np.asarray(inputs["be3"], dtype=np.float32)
    Wd1 = np.asarray(inputs["Wd1"], dtype=np.float32)
    bd1 = np.asarray(inputs["bd1"], dtype=np.float32)
    Wd2 = np.asarray(inputs["Wd2"], dtype=np.float32)
    bd2 = np.asarray(inputs["bd2"], dtype=np.float32)
    Wd3 = np.asarray(inputs["Wd3"], dtype=np.float32)
    bd3 = np.asarray(inputs["bd3"], dtype=np.float32)

    # ---- bucket rows by size ----
    idx = [np.nonzero(seq == s)[0] for s in SIZES]
    n_ks = [len(i) for i in idx]
    # even-rounded per-core counts (float32r needs even moving dims)
    c_ks = tuple(2 * (-(-n // (2 * N_CORES))) if n > 0 else 0 for n in n_ks)
    R = sum(c_ks)

    out = np.zeros((B, BASE), dtype=np.float32)
    if R == 0:
        return out

    offs = np.cumsum([0] + list(c_ks))[:-1]

    # ---- shared (replicated) weight inputs, padded to 128-multiples ----
    shared = {}
    for k in range(5):
        s = SIZES[k]
        shared[f"winT{k}"] = _pad(Win[k].T[:s, :], (SP[k], BASE_P))
        shared[f"woutT{k}"] = _pad(Wout[k].T[:, :s], (BASE_P, s))
    shared["we1T"] = _pad(We1.T, (BASE_P, H1))
    shared["we2T"] = np.ascontiguousarray(We2.T)
    shared["we3T"] = np.ascontiguousarray(We3.T)
    shared["wd1T"] = np.ascontiguousarray(Wd1.T)
    shared["wd2T"] = np.ascontiguousarray(Wd2.T)
    shared["wd3T"] = _pad(Wd3.T, (H1, BASE_P))

    bias_cols = _bias_layout()
    bp = np.zeros((128, len(bias_cols)), dtype=np.float32)
    vecs = {"L1": be1, "L2": be2, "L3": be3, "D1": bd1, "D2": bd2, "D3": bd3}
    for j, col in enumerate(bias_cols):
        layer, k, start, width = col
        if layer == "exp":
            v = bin_[k][start:start + width]
        elif layer == "out":
            v = bout[k][start:start + width]
        else:
            v = vecs[layer][start:start + width]
        bp[: len(v), j] = v
    shared["biases"] = bp

    # ---- per-core inputs ----
    in_maps = []
    core_rows = []
    for m in range(N_CORES):
        Xc = np.zeros((R, BASE_P), dtype=np.float32)
        rows_info = []
        for k in range(5):
            if c_ks[k] == 0:
                continue
            lo = m * c_ks[k]
            rows = idx[k][lo:lo + c_ks[k]]
            if len(rows):
                Xc[offs[k]:offs[k] + len(rows), :BASE] = x[rows]
            rows_info.append((k, rows, offs[k]))
        in_maps.append({"xT": np.ascontiguousarray(Xc.T), **shared})
        core_rows.append(rows_info)

    # ---- build / fetch program ----
    key = (c_ks, R)
    if key not in _prog_cache:
        _prog_cache[key] = _build_program(c_ks, R)
    nc = _prog_cache[key]

    trace = bool(os.environ.get("BASS_TRACE"))
    res = run_bass_kernel_spmd(nc, in_maps, list(range(N_CORES)), trace=trace)
    _last_exec_ns = res.exec_time_ns

    # ---- gather / unsort ----
    for m in range(N_CORES):
        oT = res.results[m]["outT"]
        for (k, rows, o) in core_rows[m]:
            if len(rows):
                out[rows] = oT[:, o:o + len(rows)].T
    return out
